# revision 4
# baseline (speedup 1.0000x reference)
"""Trainium2 Bass kernel for nn_Conv2D_BinaryLayer (3x3 VALID conv, binarized
weights, bias add).

  x      [32, 112, 112, 128] f32  (NHWC)
  kernel [3, 3, 128, 256]    f32  -> binarized on device to {-1, +1} (bf16, exact)
  bias   [256]               f32
  out    [32, 110, 110, 256] f32

Strategy: data-parallel over batch, 4 images per NeuronCore on 8 cores.
Per core, conv is an implicit GEMM: Cin=128 is exactly the PE contraction
dim. x is transposed on-chip (PE transpose) to xT[ci, pix] per image, then
each output block of <=128 pixels accumulates 9 matmuls (one per 3x3 tap)
into PSUM: psum[pix, co] += xT[ci, patch_pix].T @ wb[ci, co]. Bias is fused
into the PSUM->SBUF copy on DVE. x stays f32 until the transpose; the
PSUM->SBUF copy (ScalarE) casts activations to bf16 so conv matmuls run at
1 cycle/row.
"""

import numpy as np
from contextlib import ExitStack

import concourse.bass as bass
import concourse.tile as tile
from concourse import mybir
from concourse.bass_utils import run_bass_kernel_spmd

# ---------------------------------------------------------------- shapes
N, H, W, CIN, COUT = 32, 112, 112, 128, 256
KH = KW = 3
HO, WO = H - KH + 1, W - KW + 1  # 110, 110
N_CORES = 8
NPC = N // N_CORES               # images per core = 4
PIX = H * W                      # 12544
NT = PIX // 128                  # 98 transpose tiles per image
CHUNK_T = 14                     # transpose tiles per x-load DMA
N_CHUNK = NT // CHUNK_T          # 7 chunk DMAs per image
NTAP = KH * KW                   # 9

# Conv blocks: 128 consecutive flat positions of the 112-wide grid. All 112
# columns of each row are computed (cols 110/111 are garbage, skipped at
# store time); the matmul stationary operand stays a contiguous single-free-
# dim slice of xT, which walrus requires.
NPOS = HO * W                    # 12320 grid positions per image
NBLK = -(-NPOS // 128)           # 97 blocks
XT_PAD = PIX + 128               # tail of last block's taps reads past image


def _store_segments(s):
    """Valid output row segments for the block of positions [s, s+128):
    (partition_offset, out_row, out_col0, length), at most two."""
    segs = []
    r0, c0 = divmod(s, W)
    if r0 < HO and c0 < WO:
        segs.append((0, r0, c0, WO - c0))
    r1 = r0 + 1
    l2 = min(WO, s + 128 - W * r1)
    if r1 < HO and l2 > 0:
        segs.append((W - c0, r1, 0, l2))
    return segs

_F32 = mybir.dt.float32
_BF16 = mybir.dt.bfloat16


def _split_waits(nc, maxw=1):
    """walrus in this container rejects multiple sync-waits per instruction
    (observed on Drain and fused-LDW Matmult). Move overflow waits onto
    NoOps inserted just before the instruction — semantically identical,
    the sequencer blocks between the nop and the instruction either way."""
    def limit(inst):
        return maxw

    for f in nc.m.functions:
        for bb in f.blocks:
            new_insts = []
            for inst in bb.instructions:
                si = inst.sync_info
                mw = limit(inst)
                if si is not None and si.on_wait and len(si.on_wait) > mw:
                    waits = list(si.on_wait)
                    overflow, keep = waits[:-mw], waits[-mw:]
                    for ci in range(0, len(overflow), 1):
                        nop = mybir.InstNoOp(
                            name=f"{inst.name}-ws{ci}",
                            engine=inst.engine,
                            ins=[], outs=[],
                            sync_info=mybir.SyncInfo(
                                on_wait=overflow[ci:ci + 1], on_update=[]),
                        )
                        nc.register_instruction(nop, overwrite=True)
                        new_insts.append(nop)
                    inst.sync_info = mybir.SyncInfo(
                        on_wait=keep, on_update=list(si.on_update or []))
                new_insts.append(inst)
            bb.instructions[:] = new_insts


def build_nc():
    nc = bass.Bass("TRN2", target_bir_lowering=False, debug=False,
                   num_devices=N_CORES)

    x_d = nc.dram_tensor("x_shard", [NPC, H, W, CIN], _F32,
                         kind="ExternalInput")
    k_d = nc.dram_tensor("kern", [KH, KW, CIN, COUT], _F32,
                         kind="ExternalInput")
    b_d = nc.dram_tensor("bias_rep", [128, COUT], _F32, kind="ExternalInput")
    o_d = nc.dram_tensor("out", [NPC, HO, WO, COUT], _F32,
                         kind="ExternalOutput")

    ident = nc.inline_tensor(np.eye(128, dtype=np.float32), name="ident")

    with tile.TileContext(nc) as tc, ExitStack() as ctx:
        const_pool = ctx.enter_context(tc.tile_pool(name="const", bufs=1))
        xnat_pool = ctx.enter_context(tc.tile_pool(name="xnat", bufs=3))
        xt_pool = ctx.enter_context(tc.tile_pool(name="xt", bufs=2))
        out_pool = ctx.enter_context(tc.tile_pool(name="osb", bufs=4))
        pst_pool = ctx.enter_context(
            tc.tile_pool(name="pst", bufs=4, space="PSUM"))
        psc_pool = ctx.enter_context(
            tc.tile_pool(name="psc", bufs=4, space="PSUM"))

        # --- constants: identity, bias, binarized weights -----------------
        id_sb = const_pool.tile([128, 128], _F32, tag="ident")
        nc.sync.dma_start(id_sb[:], ident.ap()[:])

        bias_sb = const_pool.tile([128, COUT], _F32, tag="bias")
        nc.sync.dma_start(bias_sb[:], b_d.ap()[:])

        # kernel: [kh,kw,ci,co] -> SBUF [ci, (kh kw co)]
        w_f32 = const_pool.tile([128, NTAP * COUT], _F32, tag="wf32")
        k_view = k_d.ap().rearrange("kh kw ci co -> ci kh kw co")
        nc.sync.dma_start(
            w_f32[:].rearrange("p (kh kw co) -> p kh kw co", kh=KH, kw=KW),
            k_view)
        # binarize, exactly matching fp32 ref semantics:
        #   wb = +1  iff  fl(w + 1.0) > 1.0  else -1
        cmp = const_pool.tile([128, NTAP * COUT], _F32, tag="cmp")
        nc.vector.tensor_scalar(cmp[:], w_f32[:], 1.0, 1.0,
                                mybir.AluOpType.add, mybir.AluOpType.is_gt)
        wb = const_pool.tile([128, NTAP * COUT], _BF16, tag="wb")
        nc.vector.tensor_scalar(wb[:], cmp[:], 2.0, 1.0,
                                mybir.AluOpType.mult,
                                mybir.AluOpType.subtract)

        # x as flat pixel-major view: [(n h w), c] -> [p, t, c] tiles
        x_flat = x_d.ap().rearrange("n h w c -> (n h w) c")
        x_tiled = x_flat.rearrange("(t p) c -> p t c", p=128)  # t = NPC*NT

        for n in range(NPC):
            # ---- load + transpose one image into xT [ci, h*W+w] (bf16)
            xt = xt_pool.tile([128, XT_PAD], _BF16, tag="xt")
            nc.vector.memset(xt[:, PIX:XT_PAD], 0.0)
            for j in range(N_CHUNK):
                xn = xnat_pool.tile([128, CHUNK_T * 128], _F32, tag="xnat")
                t0 = n * NT + j * CHUNK_T
                nc.sync.dma_start(
                    xn[:].rearrange("p (t c) -> p t c", c=128),
                    x_tiled[:, t0:t0 + CHUNK_T, :])
                for k in range(CHUNK_T):
                    pst = pst_pool.tile([128, 128], _F32, tag="pst")
                    nc.tensor.transpose(
                        pst[:], xn[:, k * 128:(k + 1) * 128], id_sb[:])
                    pos = (j * CHUNK_T + k) * 128
                    nc.scalar.copy(xt[:, pos:pos + 128], pst[:])

            # ---- conv: NBLK output blocks x 9 taps
            for b in range(NBLK):
                s = 128 * b
                psc = psc_pool.tile([128, COUT], _F32, tag="psc")
                for tap in range(NTAP):
                    off = s + (tap // KW) * W + (tap % KW)
                    nc.tensor.matmul(
                        psc[:, :], xt[:, off:off + 128],
                        wb[:, tap * COUT:(tap + 1) * COUT],
                        start=(tap == 0), stop=(tap == NTAP - 1))
                osb = out_pool.tile([128, COUT], _F32, tag="osb")
                nc.vector.tensor_add(osb[:], psc[:], bias_sb[:])
                for (poff, r, c0, length) in _store_segments(s):
                    nc.sync.dma_start(
                        o_d.ap()[n, r, c0:c0 + length, :],
                        osb[poff:poff + length, :])

    _split_waits(nc)
    return nc


_NC_CACHE = None


def _get_nc():
    global _NC_CACHE
    if _NC_CACHE is None:
        _NC_CACHE = build_nc()
    return _NC_CACHE


def kernel(x: np.ndarray, kernel: np.ndarray, bias: np.ndarray) -> np.ndarray:
    nc = _get_nc()
    bias_rep = np.ascontiguousarray(
        np.broadcast_to(bias.astype(np.float32), (128, COUT)))
    in_maps = [
        {
            "x_shard": np.ascontiguousarray(x[c * NPC:(c + 1) * NPC]),
            "kern": np.ascontiguousarray(kernel.astype(np.float32)),
            "bias_rep": bias_rep,
        }
        for c in range(N_CORES)
    ]
    res = run_bass_kernel_spmd(nc, in_maps, list(range(N_CORES)))
    out = np.concatenate([res.results[c]["out"] for c in range(N_CORES)],
                         axis=0)
    return out.astype(np.float32)


# revision 9
# speedup vs baseline: 1.0309x; 1.0309x over previous
"""Trainium2 Bass kernel for nn_Conv2D_BinaryLayer (3x3 VALID conv, binarized
weights, bias add).

  x      [32, 112, 112, 128] f32  (NHWC)
  kernel [3, 3, 128, 256]    f32  -> binarized on device to {-1, +1} (bf16, exact)
  bias   [256]               f32
  out    [32, 110, 110, 256] f32

Strategy: data-parallel over batch, 4 images per NeuronCore on 8 cores.
Per core, conv is an implicit GEMM: Cin=128 is exactly the PE contraction
dim. x is transposed on-chip (PE transpose) to xT[ci, pix] per image, then
each output block of <=128 pixels accumulates 9 matmuls (one per 3x3 tap)
into PSUM: psum[pix, co] += xT[ci, patch_pix].T @ wb[ci, co]. Bias is fused
into the PSUM->SBUF copy on DVE. x stays f32 until the transpose; the
PSUM->SBUF copy (ScalarE) casts activations to bf16 so conv matmuls run at
1 cycle/row.
"""

import numpy as np
from contextlib import ExitStack

import concourse.bass as bass
import concourse.tile as tile
from concourse import mybir
from concourse.bass_utils import run_bass_kernel_spmd

# ---------------------------------------------------------------- shapes
N, H, W, CIN, COUT = 32, 112, 112, 128, 256
KH = KW = 3
HO, WO = H - KH + 1, W - KW + 1  # 110, 110
N_CORES = 8
NPC = N // N_CORES               # images per core = 4
PIX = H * W                      # 12544
NT = PIX // 128                  # 98 transpose tiles per image
CHUNK_T = 14                     # transpose tiles per x-load DMA
N_CHUNK = NT // CHUNK_T          # 7 chunk DMAs per image
NTAP = KH * KW                   # 9

# Conv blocks: 128 consecutive flat positions of the 112-wide grid. All 112
# columns of each row are computed (cols 110/111 are garbage, skipped at
# store time); the matmul stationary operand stays a contiguous single-free-
# dim slice of xT, which walrus requires.
NPOS = HO * W                    # 12320 grid positions per image
NBLK = -(-NPOS // 128)           # 97 blocks
XT_PAD = PIX + 128               # tail of last block's taps reads past image


def _store_segments(s):
    """Valid output row segments for the block of positions [s, s+128):
    (partition_offset, out_row, out_col0, length), at most two."""
    segs = []
    r0, c0 = divmod(s, W)
    if r0 < HO and c0 < WO:
        segs.append((0, r0, c0, WO - c0))
    r1 = r0 + 1
    l2 = min(WO, s + 128 - W * r1)
    if r1 < HO and l2 > 0:
        segs.append((W - c0, r1, 0, l2))
    return segs

_F32 = mybir.dt.float32
_BF16 = mybir.dt.bfloat16


def _split_waits(nc, maxw=1):
    """walrus in this container rejects multiple sync-waits per instruction
    (observed on Drain and fused-LDW Matmult). Move overflow waits onto
    NoOps inserted just before the instruction — semantically identical,
    the sequencer blocks between the nop and the instruction either way."""
    def limit(inst):
        return maxw

    for f in nc.m.functions:
        for bb in f.blocks:
            new_insts = []
            for inst in bb.instructions:
                si = inst.sync_info
                mw = limit(inst)
                if si is not None and si.on_wait and len(si.on_wait) > mw:
                    waits = list(si.on_wait)
                    overflow, keep = waits[:-mw], waits[-mw:]
                    for ci in range(0, len(overflow), 1):
                        nop = mybir.InstNoOp(
                            name=f"{inst.name}-ws{ci}",
                            engine=inst.engine,
                            ins=[], outs=[],
                            sync_info=mybir.SyncInfo(
                                on_wait=overflow[ci:ci + 1], on_update=[]),
                        )
                        nc.register_instruction(nop, overwrite=True)
                        new_insts.append(nop)
                    inst.sync_info = mybir.SyncInfo(
                        on_wait=keep, on_update=list(si.on_update or []))
                new_insts.append(inst)
            bb.instructions[:] = new_insts


def build_nc():
    nc = bass.Bass("TRN2", target_bir_lowering=False, debug=False,
                   num_devices=N_CORES)

    x_d = nc.dram_tensor("x_shard", [NPC, H, W, CIN], _F32,
                         kind="ExternalInput")
    k_d = nc.dram_tensor("kern", [KH, KW, CIN, COUT], _F32,
                         kind="ExternalInput")
    b_d = nc.dram_tensor("bias_rep", [128, COUT], _F32, kind="ExternalInput")
    o_d = nc.dram_tensor("out", [NPC, HO, WO, COUT], _F32,
                         kind="ExternalOutput")

    import ml_dtypes
    ident = nc.inline_tensor(np.eye(128, dtype=ml_dtypes.bfloat16),
                             name="ident")

    with tile.TileContext(nc) as tc, ExitStack() as ctx:
        const_pool = ctx.enter_context(tc.tile_pool(name="const", bufs=1))
        xnat_pool = ctx.enter_context(tc.tile_pool(name="xnat", bufs=3))
        xt_pool = ctx.enter_context(tc.tile_pool(name="xt", bufs=2))
        out_pool = ctx.enter_context(tc.tile_pool(name="osb", bufs=4))
        pst_pool = ctx.enter_context(
            tc.tile_pool(name="pst", bufs=4, space="PSUM"))
        psc_pool = ctx.enter_context(
            tc.tile_pool(name="psc", bufs=4, space="PSUM"))

        # --- constants: identity, bias, binarized weights -----------------
        id_sb = const_pool.tile([128, 128], _BF16, tag="ident")
        nc.sync.dma_start(id_sb[:], ident.ap()[:])

        bias_sb = const_pool.tile([128, COUT], _F32, tag="bias")
        nc.sync.dma_start(bias_sb[:], b_d.ap()[:])

        # kernel: [kh,kw,ci,co] -> SBUF [ci, (kh kw co)]
        w_f32 = const_pool.tile([128, NTAP * COUT], _F32, tag="wf32")
        k_view = k_d.ap().rearrange("kh kw ci co -> ci kh kw co")
        nc.sync.dma_start(
            w_f32[:].rearrange("p (kh kw co) -> p kh kw co", kh=KH, kw=KW),
            k_view)
        # binarize, exactly matching fp32 ref semantics:
        #   wb = +1  iff  fl(w + 1.0) > 1.0  else -1
        cmp = const_pool.tile([128, NTAP * COUT], _F32, tag="cmp")
        nc.vector.tensor_scalar(cmp[:], w_f32[:], 1.0, 1.0,
                                mybir.AluOpType.add, mybir.AluOpType.is_gt)
        wb = const_pool.tile([128, NTAP * COUT], _BF16, tag="wb")
        nc.vector.tensor_scalar(wb[:], cmp[:], 2.0, 1.0,
                                mybir.AluOpType.mult,
                                mybir.AluOpType.subtract)

        # x as flat pixel-major view: [(n h w), c] -> [p, t, c] tiles
        x_flat = x_d.ap().rearrange("n h w c -> (n h w) c")
        x_tiled = x_flat.rearrange("(t p) c -> p t c", p=128)  # t = NPC*NT

        for n in range(NPC):
            # ---- load + transpose one image into xT [ci, h*W+w] (bf16)
            xt = xt_pool.tile([128, XT_PAD], _BF16, tag="xt")
            nc.vector.memset(xt[:, PIX:XT_PAD], 0.0)
            for j in range(N_CHUNK):
                # loads ride the ACT HWDGE ring so they never queue behind
                # the output stores (SP ring)
                xn = xnat_pool.tile([128, CHUNK_T * 128], _F32, tag="xnat")
                t0 = n * NT + j * CHUNK_T
                nc.scalar.dma_start(
                    xn[:].rearrange("p (t c) -> p t c", c=128),
                    x_tiled[:, t0:t0 + CHUNK_T, :])
                # bf16 cast (DVE 2x mode): PE transpose of bf16 runs 2x
                # faster than f32
                xnb = xnat_pool.tile([128, CHUNK_T * 128], _BF16, tag="xnatb")
                nc.vector.tensor_copy(xnb[:], xn[:])
                for k in range(CHUNK_T):
                    pst = pst_pool.tile([128, 128], _BF16, tag="pst")
                    nc.tensor.transpose(
                        pst[:], xnb[:, k * 128:(k + 1) * 128], id_sb[:])
                    pos = (j * CHUNK_T + k) * 128
                    nc.scalar.copy(xt[:, pos:pos + 128], pst[:])

            # ---- conv: NBLK output blocks x 9 taps
            for b in range(NBLK):
                s = 128 * b
                psc = psc_pool.tile([128, COUT], _F32, tag="psc")
                for tap in range(NTAP):
                    off = s + (tap // KW) * W + (tap % KW)
                    nc.tensor.matmul(
                        psc[:, :], xt[:, off:off + 128],
                        wb[:, tap * COUT:(tap + 1) * COUT],
                        start=(tap == 0), stop=(tap == NTAP - 1))
                osb = out_pool.tile([128, COUT], _F32, tag="osb")
                nc.vector.tensor_add(osb[:], psc[:], bias_sb[:])
                for (poff, r, c0, length) in _store_segments(s):
                    nc.sync.dma_start(
                        o_d.ap()[n, r, c0:c0 + length, :],
                        osb[poff:poff + length, :])

    _split_waits(nc)
    return nc


_NC_CACHE = None


def _get_nc():
    global _NC_CACHE
    if _NC_CACHE is None:
        _NC_CACHE = build_nc()
    return _NC_CACHE


def kernel(x: np.ndarray, kernel: np.ndarray, bias: np.ndarray) -> np.ndarray:
    nc = _get_nc()
    bias_rep = np.ascontiguousarray(
        np.broadcast_to(bias.astype(np.float32), (128, COUT)))
    in_maps = [
        {
            "x_shard": np.ascontiguousarray(x[c * NPC:(c + 1) * NPC]),
            "kern": np.ascontiguousarray(kernel.astype(np.float32)),
            "bias_rep": bias_rep,
        }
        for c in range(N_CORES)
    ]
    res = run_bass_kernel_spmd(nc, in_maps, list(range(N_CORES)))
    out = np.concatenate([res.results[c]["out"] for c in range(N_CORES)],
                         axis=0)
    return out.astype(np.float32)


# revision 11
# speedup vs baseline: 1.0952x; 1.0624x over previous
"""Trainium2 Bass kernel for nn_Conv2D_BinaryLayer (3x3 VALID conv, binarized
weights, bias add).

  x      [32, 112, 112, 128] f32  (NHWC)
  kernel [3, 3, 128, 256]    f32  -> binarized on device to {-1, +1} (bf16, exact)
  bias   [256]               f32
  out    [32, 110, 110, 256] f32

Strategy: data-parallel over batch, 4 images per NeuronCore on 8 cores.
Per core, conv is an implicit GEMM: Cin=128 is exactly the PE contraction
dim. x is transposed on-chip (PE transpose) to xT[ci, pix] per image, then
each output block of <=128 pixels accumulates 9 matmuls (one per 3x3 tap)
into PSUM: psum[pix, co] += xT[ci, patch_pix].T @ wb[ci, co]. Bias is fused
into the PSUM->SBUF copy on DVE. x stays f32 until the transpose; the
PSUM->SBUF copy (ScalarE) casts activations to bf16 so conv matmuls run at
1 cycle/row.
"""

import numpy as np
from contextlib import ExitStack

import concourse.bass as bass
import concourse.tile as tile
from concourse import mybir
from concourse.bass_utils import run_bass_kernel_spmd

# ---------------------------------------------------------------- shapes
N, H, W, CIN, COUT = 32, 112, 112, 128, 256
KH = KW = 3
HO, WO = H - KH + 1, W - KW + 1  # 110, 110
N_CORES = 8
NPC = N // N_CORES               # images per core = 4
PIX = H * W                      # 12544
NT = PIX // 128                  # 98 transpose tiles per image
CHUNK_T = 14                     # transpose tiles per x-load DMA
N_CHUNK = NT // CHUNK_T          # 7 chunk DMAs per image
NTAP = KH * KW                   # 9

# Conv blocks: 128 consecutive flat positions of the 112-wide grid. All 112
# columns of each row are computed (cols 110/111 are garbage, skipped at
# store time); the matmul stationary operand stays a contiguous single-free-
# dim slice of xT, which walrus requires.
NPOS = HO * W                    # 12320 grid positions per image
NBLK = -(-NPOS // 128)           # 97 blocks
XT_PAD = PIX + 128               # tail of last block's taps reads past image


def _store_segments(s):
    """Valid output row segments for the block of positions [s, s+128):
    (partition_offset, out_row, out_col0, length), at most two."""
    segs = []
    r0, c0 = divmod(s, W)
    if r0 < HO and c0 < WO:
        segs.append((0, r0, c0, WO - c0))
    r1 = r0 + 1
    l2 = min(WO, s + 128 - W * r1)
    if r1 < HO and l2 > 0:
        segs.append((W - c0, r1, 0, l2))
    return segs

_F32 = mybir.dt.float32
_BF16 = mybir.dt.bfloat16


def _split_waits(nc, maxw=1):
    """walrus in this container rejects multiple sync-waits per instruction
    (observed on Drain and fused-LDW Matmult). Move overflow waits onto
    NoOps inserted just before the instruction — semantically identical,
    the sequencer blocks between the nop and the instruction either way."""
    def limit(inst):
        return maxw

    for f in nc.m.functions:
        for bb in f.blocks:
            new_insts = []
            for inst in bb.instructions:
                si = inst.sync_info
                mw = limit(inst)
                if si is not None and si.on_wait and len(si.on_wait) > mw:
                    waits = list(si.on_wait)
                    overflow, keep = waits[:-mw], waits[-mw:]
                    for ci in range(0, len(overflow), 1):
                        nop = mybir.InstNoOp(
                            name=f"{inst.name}-ws{ci}",
                            engine=inst.engine,
                            ins=[], outs=[],
                            sync_info=mybir.SyncInfo(
                                on_wait=overflow[ci:ci + 1], on_update=[]),
                        )
                        nc.register_instruction(nop, overwrite=True)
                        new_insts.append(nop)
                    inst.sync_info = mybir.SyncInfo(
                        on_wait=keep, on_update=list(si.on_update or []))
                new_insts.append(inst)
            bb.instructions[:] = new_insts


def build_nc():
    nc = bass.Bass("TRN2", target_bir_lowering=False, debug=False,
                   num_devices=N_CORES, num_swdge_queues=2)

    x_d = nc.dram_tensor("x_shard", [NPC, H, W, CIN], _F32,
                         kind="ExternalInput")
    k_d = nc.dram_tensor("kern", [KH, KW, CIN, COUT], _F32,
                         kind="ExternalInput")
    b_d = nc.dram_tensor("bias_rep", [128, COUT], _F32, kind="ExternalInput")
    o_d = nc.dram_tensor("out", [NPC, HO, WO, COUT], _F32,
                         kind="ExternalOutput")

    import ml_dtypes
    ident = nc.inline_tensor(np.eye(128, dtype=ml_dtypes.bfloat16),
                             name="ident")

    with tile.TileContext(nc) as tc, ExitStack() as ctx:
        const_pool = ctx.enter_context(tc.tile_pool(name="const", bufs=1))
        xnat_pool = ctx.enter_context(tc.tile_pool(name="xnat", bufs=3))
        xt_pool = ctx.enter_context(tc.tile_pool(name="xt", bufs=2))
        out_pool = ctx.enter_context(tc.tile_pool(name="osb", bufs=8))
        pst_pool = ctx.enter_context(
            tc.tile_pool(name="pst", bufs=2, space="PSUM"))
        psc_pool = ctx.enter_context(
            tc.tile_pool(name="psc", bufs=6, space="PSUM"))

        # --- constants: identity, bias, binarized weights -----------------
        id_sb = const_pool.tile([128, 128], _BF16, tag="ident")
        nc.sync.dma_start(id_sb[:], ident.ap()[:])

        bias_sb = const_pool.tile([128, COUT], _F32, tag="bias")
        nc.sync.dma_start(bias_sb[:], b_d.ap()[:])

        # kernel: [kh,kw,ci,co] -> SBUF [ci, (kh kw co)]
        w_f32 = const_pool.tile([128, NTAP * COUT], _F32, tag="wf32")
        k_view = k_d.ap().rearrange("kh kw ci co -> ci kh kw co")
        nc.sync.dma_start(
            w_f32[:].rearrange("p (kh kw co) -> p kh kw co", kh=KH, kw=KW),
            k_view)
        # binarize, exactly matching fp32 ref semantics:
        #   wb = +1  iff  fl(w + 1.0) > 1.0  else -1
        cmp = const_pool.tile([128, NTAP * COUT], _F32, tag="cmp")
        nc.vector.tensor_scalar(cmp[:], w_f32[:], 1.0, 1.0,
                                mybir.AluOpType.add, mybir.AluOpType.is_gt)
        wb = const_pool.tile([128, NTAP * COUT], _BF16, tag="wb")
        nc.vector.tensor_scalar(wb[:], cmp[:], 2.0, 1.0,
                                mybir.AluOpType.mult,
                                mybir.AluOpType.subtract)

        # x as flat pixel-major view: [(n h w), c] -> [p, t, c] tiles
        x_flat = x_d.ap().rearrange("n h w c -> (n h w) c")
        x_tiled = x_flat.rearrange("(t p) c -> p t c", p=128)  # t = NPC*NT

        for n in range(NPC):
            # ---- load + transpose one image into xT [ci, h*W+w] (bf16)
            xt = xt_pool.tile([128, XT_PAD], _BF16, tag="xt")
            nc.vector.memset(xt[:, PIX:XT_PAD], 0.0)
            for j in range(N_CHUNK):
                # loads ride the ACT HWDGE ring so they never queue behind
                # the output stores (SP ring)
                xn = xnat_pool.tile([128, CHUNK_T * 128], _F32, tag="xnat")
                t0 = n * NT + j * CHUNK_T
                nc.scalar.dma_start(
                    xn[:].rearrange("p (t c) -> p t c", c=128),
                    x_tiled[:, t0:t0 + CHUNK_T, :])
                # bf16 cast (DVE 2x mode): PE transpose of bf16 runs 2x
                # faster than f32
                xnb = xnat_pool.tile([128, CHUNK_T * 128], _BF16, tag="xnatb")
                nc.vector.tensor_copy(xnb[:], xn[:])
                for k in range(CHUNK_T):
                    pst = pst_pool.tile([128, 128], _BF16, tag="pst")
                    nc.tensor.transpose(
                        pst[:], xnb[:, k * 128:(k + 1) * 128], id_sb[:])
                    pos = (j * CHUNK_T + k) * 128
                    nc.scalar.copy(xt[:, pos:pos + 128], pst[:])

            # ---- conv: NBLK output blocks x 9 taps
            for b in range(NBLK):
                s = 128 * b
                psc = psc_pool.tile([128, COUT], _F32, tag="psc")
                for tap in range(NTAP):
                    off = s + (tap // KW) * W + (tap % KW)
                    nc.tensor.matmul(
                        psc[:, :], xt[:, off:off + 128],
                        wb[:, tap * COUT:(tap + 1) * COUT],
                        start=(tap == 0), stop=(tap == NTAP - 1))
                osb = out_pool.tile([128, COUT], _F32, tag="osb")
                nc.vector.tensor_add(osb[:], psc[:], bias_sb[:])
                for (poff, r, c0, length) in _store_segments(s):
                    nc.gpsimd.dma_start(
                        o_d.ap()[n, r, c0:c0 + length, :],
                        osb[poff:poff + length, :])

    _split_waits(nc)
    return nc


_NC_CACHE = None


def _get_nc():
    global _NC_CACHE
    if _NC_CACHE is None:
        _NC_CACHE = build_nc()
    return _NC_CACHE


def kernel(x: np.ndarray, kernel: np.ndarray, bias: np.ndarray) -> np.ndarray:
    nc = _get_nc()
    bias_rep = np.ascontiguousarray(
        np.broadcast_to(bias.astype(np.float32), (128, COUT)))
    in_maps = [
        {
            "x_shard": np.ascontiguousarray(x[c * NPC:(c + 1) * NPC]),
            "kern": np.ascontiguousarray(kernel.astype(np.float32)),
            "bias_rep": bias_rep,
        }
        for c in range(N_CORES)
    ]
    res = run_bass_kernel_spmd(nc, in_maps, list(range(N_CORES)))
    out = np.concatenate([res.results[c]["out"] for c in range(N_CORES)],
                         axis=0)
    return out.astype(np.float32)
